# revision 1
# baseline (speedup 1.0000x reference)
"""sqllm 4-bit LUT-quantized linear: y = x @ dequant(qweight, lut).T
Trainium2 Bass kernel, 8 NeuronCores, data-parallel (shard tokens).

Per core c (TL = 1024 tokens):
  - x_c arrives IN the donated output buffer xy [1024, 4096] f32 (x and y
    have the same per-core shape since K == N == 4096). No zeros upload,
    no input replication: H2D is x (128 MB, sharded) + qt/coef (~66 MB
    replicated); D2H is y (128 MB) assembled by shard_map out_specs.
  - Phase A: PE-transpose x_c -> xt [k'', t] bf16 SBUF-resident, where
    k'' = j*512 + i (nibble-plane-major) is selected by stride-8 reads
    of the natural k = 8i + j layout.
  - Phase B+C (hw loop over 8 n-groups of 512): dequant W[n, k''] via
    3 bit-extracts + int->fp + 4 half-cubics (ACT) + 4 fused cubic-tails
    (custom DVE) + 3 predicated merges; PE-transpose -> W^T bf16; then
    psum[t128, n512] += xt_tile.T @ W^T_tile over 32 k-tiles; y chunk
    DMAs back into xy (row tt only overlaps x rows already consumed).
"""

import hashlib
import os
import pickle

import numpy as np
import jax
from jax.sharding import Mesh, PartitionSpec
from jax.experimental.shard_map import shard_map

import concourse.bass as bass
import concourse.mybir as mybir
import concourse.tile as tile
from concourse import bacc
from concourse.bass2jax import (
    _bass_exec_p,
    install_neuronx_cc_hook,
    partition_id_tensor,
)
from concourse.masks import make_identity

# ---------------- problem constants (hardcoded per contract) ---------------- #
B, S, K, N = 4, 2048, 4096, 4096
T = B * S                 # 8192 tokens
NCORES = 8
TL = T // NCORES          # 1024 tokens per core
KT = K // 128             # 32 contraction tiles
IW = K // 8               # 512 packed int32 rows
NGR = 8                   # n-groups per core
NG = N // NGR             # 512 out features per group
TT = TL // 128            # 8 token tiles per core
LOOP_R = 1                # timing: repeat whole device program
NEFF_CACHE = os.path.expanduser("~/.cache/bass_neff_cache")

F32 = mybir.dt.float32
BF16 = mybir.dt.bfloat16
I32 = mybir.dt.int32

# ---------------- custom DVE op: cubic tail ---------------- #
_CUBIC = None


def _register_cubic_tail():
    """out = s0 + in0*s1 + in0^2 * in1   (s0,s1 per-partition scalars)"""
    global _CUBIC
    if _CUBIC is not None:
        return _CUBIC
    from concourse.dve_ops import DveOp, OPS, CUSTOM_DVE_SPECS, _SUB_OPCODE_FOR_NAME
    from concourse.dve_spec import Spec, Src0, Src1, C0, C1, sq, lower as dve_lower
    from concourse.dve_uop import DveOpSpec

    name = "SQLLM_CUBIC_TAIL"
    if name in _SUB_OPCODE_FOR_NAME:
        _CUBIC = next(op for op in OPS if op.name == name)
        return _CUBIC
    spec = Spec(
        body=C0 + Src0 * C1 + sq(Src0) * Src1,
        reference=lambda in0, in1, s0, s1, imm2: (
            s0 + in0 * s1 + in0 * in0 * in1
        ).astype(np.float32),
    )
    shas = {}
    for ver in ("v3", "v4"):
        tmp = DveOpSpec(name=name, opcode=1, uops=dve_lower(spec, ver=ver), rd1_en=True)
        shas[ver] = tmp.sha(ver)
    op = DveOp(name, spec, subdim=False, uops_sha=shas)
    row = max(_SUB_OPCODE_FOR_NAME.values()) + 1
    assert row < 0x20
    OPS.append(op)
    CUSTOM_DVE_SPECS[name] = spec
    _SUB_OPCODE_FOR_NAME[name] = row
    _CUBIC = op
    return op


# ---------------- device program ---------------- #
def build_nc():
    CUBIC = _register_cubic_tail()
    nc = bacc.Bacc("TRN2", target_bir_lowering=False)
    qt = nc.dram_tensor("qt", [N + 512, IW], I32, kind="ExternalInput")
    coef = nc.dram_tensor("coef", [N + 512, 16], F32, kind="ExternalInput")
    xy = nc.dram_tensor("xy", [TL, K], BF16, kind="ExternalOutput")

    qt_v = qt.rearrange("(nt p) i -> p nt i", p=128)       # [128, 32, IW]
    coef_v = coef.rearrange("(nt p) c -> p nt c", p=128)   # [128, 32, 16]
    xy_v = xy.rearrange("(tt p) k -> p tt k", p=128)       # [128, TT, K]

    with tile.TileContext(nc) as tc:
        with (
            tc.tile_pool(name="persist", bufs=1) as persist,
            tc.tile_pool(name="xt", bufs=1) as xtp,
            tc.tile_pool(name="xs", bufs=2) as xsp,
            tc.tile_pool(name="wt", bufs=1) as wtp,
            tc.tile_pool(name="q", bufs=2) as qp,
            tc.tile_pool(name="c", bufs=2) as cp,
            tc.tile_pool(name="dq", bufs=2) as dqp,
            tc.tile_pool(name="wn", bufs=2) as wnp,
            tc.tile_pool(name="yb", bufs=3) as ybp,
            tc.tile_pool(name="pst", bufs=2, space="PSUM") as pstp,
            tc.tile_pool(name="ps", bufs=4, space="PSUM") as psp,
        ):
            ident = persist.tile([128, 128], F32, tag="ident")
            make_identity(nc, ident[:])
            identb = persist.tile([128, 128], BF16, tag="identb")
            nc.vector.tensor_copy(identb[:], ident[:])
            # xt: 32 k''-tiles side by side, each [128 k, TL t] bf16
            xt_all = xtp.tile([128, KT * TL], BF16, tag="xt")
            xt_v = xt_all[:].rearrange("p (s t) -> p s t", t=TL)
            # W^T for one n-group: 32 k''-tiles of [128 k, NG n] bf16
            wt_a = wtp.tile([128, KT * NG], BF16, tag="wta")
            wt_va = wt_a[:].rearrange("p (s n) -> p s n", n=NG)
            wt_b = wtp.tile([128, KT * NG], BF16, tag="wtb")
            wt_vb = wt_b[:].rearrange("p (s n) -> p s n", n=NG)

            def emit_all():
                # ---- Phase A: transpose x (strided k''-selection) ----
                for tt in range(TT):
                    xst = xsp.tile([128, K], BF16, tag="xs")
                    nc.sync.dma_start(xst[:], xy_v[:, tt, :])
                    vv = xst[:].rearrange("p (i j) -> p j i", j=8)  # [128,8,512]
                    for j in range(8):
                        pst = pstp.tile([128, 512], BF16)
                        for r in range(4):
                            nc.tensor.transpose(
                                pst[:, r * 128:(r + 1) * 128],
                                vv[:, j, r * 128:(r + 1) * 128],
                                identb[:],
                            )
                        nc.vector.tensor_copy(
                            xt_v[:, 4 * j:4 * j + 4, tt * 128:tt * 128 + 128],
                            pst[:].rearrange("p (r t) -> p r t", t=128),
                        )

                # ---- Phase B+C: software-pipelined hw loop over n-group pairs ----
                def emit_deq(gofs, wt_v):
                    q_sb = qp.tile([128, 4 * IW], I32, tag="q", name="q_sb")
                    nc.sync.dma_start(
                        q_sb[:].rearrange("p (nt i) -> p nt i", nt=4),
                        qt_v[:, bass.ds(gofs, 4), :],
                    )
                    c_sb = cp.tile([128, 64], F32, tag="c", name="c_sb")
                    nc.sync.dma_start(
                        c_sb[:].rearrange("p (nt c) -> p nt c", nt=4),
                        coef_v[:, bass.ds(gofs, 4), :],
                    )
                    for nt in range(4):
                        q = q_sb[:, nt * IW:(nt + 1) * IW]
                        c = c_sb[:, nt * 16:(nt + 1) * 16]
                        for j in range(8):
                            lo2i = dqp.tile([128, IW], I32, tag="lo2i")
                            nc.vector.tensor_scalar(
                                out=lo2i[:], in0=q, scalar1=4 * j, scalar2=3,
                                op0=mybir.AluOpType.logical_shift_right,
                                op1=mybir.AluOpType.bitwise_and,
                            )
                            bh = dqp.tile([128, IW], I32, tag="bh")
                            nc.vector.tensor_scalar(
                                out=bh[:], in0=q, scalar1=4 * j, scalar2=4,
                                op0=mybir.AluOpType.logical_shift_right,
                                op1=mybir.AluOpType.bitwise_and,
                            )
                            BH = dqp.tile([128, IW], I32, tag="BH")
                            nc.vector.tensor_scalar(
                                out=BH[:], in0=q, scalar1=4 * j, scalar2=8,
                                op0=mybir.AluOpType.logical_shift_right,
                                op1=mybir.AluOpType.bitwise_and,
                            )
                            lo2f = dqp.tile([128, IW], F32, tag="lo2f")
                            nc.scalar.copy(lo2f[:], lo2i[:])
                            Us = [
                                wnp.tile([128, IW], F32, tag=f"U{m}", name=f"U{m}")
                                for m in range(4)
                            ]
                            for gi in range(4):
                                half = dqp.tile([128, IW], F32, tag="h")
                                nc.scalar.activation(
                                    half[:], lo2f[:],
                                    mybir.ActivationFunctionType.Identity,
                                    bias=c[:, 4 * gi + 2: 4 * gi + 3],
                                    scale=c[:, 4 * gi + 3: 4 * gi + 4],
                                )
                                nc.vector._custom_dve(
                                    CUBIC, out=Us[gi][:], in0=lo2f[:], in1=half[:],
                                    s0=c[:, 4 * gi: 4 * gi + 1],
                                    s1=c[:, 4 * gi + 1: 4 * gi + 2],
                                )
                            nc.vector.copy_predicated(Us[0][:], bh[:], Us[1][:])
                            nc.vector.copy_predicated(Us[2][:], bh[:], Us[3][:])
                            nc.vector.copy_predicated(Us[0][:], BH[:], Us[2][:])
                            pst = pstp.tile([128, 512], F32)
                            for r in range(4):
                                nc.tensor.transpose(
                                    pst[:, r * 128:(r + 1) * 128],
                                    Us[0][:, r * 128:(r + 1) * 128],
                                    ident[:],
                                )
                            nc.vector.tensor_copy(
                                wt_v[:, 4 * j:4 * j + 4, nt * 128:nt * 128 + 128],
                                pst[:].rearrange("p (r n) -> p r n", n=128),
                            )

                def emit_mm(nofs, wt_v):
                    for tt in range(TT):
                        ps = psp.tile([128, NG], F32)
                        for s in range(KT):
                            nc.tensor.matmul(
                                ps[:],
                                xt_v[:, s, tt * 128:tt * 128 + 128],
                                wt_v[:, s, :],
                                start=(s == 0),
                                stop=(s == KT - 1),
                            )
                        yb = ybp.tile([128, NG], BF16, tag="yb")
                        nc.vector.tensor_copy(yb[:], ps[:])
                        nc.sync.dma_start(
                            xy_v[:, tt, bass.ds(nofs, NG)], yb[:]
                        )

                emit_deq(0, wt_va)
                with tc.For_i(0, NGR, 2) as g:
                    emit_deq(g * 4 + 4, wt_vb)   # dequant group g+1 (DVE/ACT)
                    emit_mm(g * NG, wt_va)       # matmuls group g (PE) overlap
                    emit_deq(g * 4 + 8, wt_va)   # dequant g+2 (pad rows at tail)
                    emit_mm(g * NG + NG, wt_vb)  # matmuls group g+1

            if LOOP_R > 1:
                with tc.For_i(0, LOOP_R, 1) as _i:
                    emit_all()
            else:
                emit_all()
    nc.compile()
    return nc


_NC_CACHE = None


def _get_nc():
    global _NC_CACHE
    if _NC_CACHE is None:
        _NC_CACHE = build_nc()
    return _NC_CACHE


# ---------------- NEFF disk cache (content-addressed) ---------------- #
_CACHE_INSTALLED = False


def _install_neff_cache():
    global _CACHE_INSTALLED
    install_neuronx_cc_hook()
    if _CACHE_INSTALLED:
        return
    try:
        import libneuronxla
    except ImportError:
        return
    inner = libneuronxla.neuronx_cc

    def cached_cc(code, code_format, platform_version, file_prefix):
        if b"bass_exec" not in code:
            return inner(code, code_format, platform_version, file_prefix)
        key = hashlib.sha256(
            code + bytes(code_format) + str(platform_version).encode()
        ).hexdigest()
        path = os.path.join(NEFF_CACHE, key + ".pkl")
        try:
            with open(path, "rb") as f:
                return pickle.load(f)
        except Exception:
            pass
        r = inner(code, code_format, platform_version, file_prefix)
        try:
            os.makedirs(NEFF_CACHE, exist_ok=True)
            tmp = path + f".tmp{os.getpid()}"
            with open(tmp, "wb") as f:
                pickle.dump(r, f)
            os.replace(tmp, path)
        except Exception:
            pass
        return r

    libneuronxla.neuronx_cc = cached_cc
    _CACHE_INSTALLED = True


# ---------------- host-side prep ---------------- #
_VINV = np.linalg.inv(np.vander(np.arange(4.0), increasing=True)).astype(np.float64)


_CPU = None
_TO_BF16 = None
_TO_F32 = None


def _cpu_casts():
    global _CPU, _TO_BF16, _TO_F32
    if _CPU is None:
        import jax.numpy as jnp
        _CPU = jax.local_devices(backend="cpu")[0]

        def _mk(dt):
            def f(a):
                return a.astype(dt)
            return f
        with jax.default_device(_CPU):
            _TO_BF16 = jax.jit(_mk(jnp.bfloat16))
            _TO_F32 = jax.jit(_mk(jnp.float32))
    return _TO_BF16, _TO_F32


def _host_prep(input, qweight, lut):
    to_bf16, _ = _cpu_casts()
    with jax.default_device(_CPU):
        x2d = np.asarray(
            to_bf16(np.asarray(input, dtype=np.float32).reshape(T, K))
        )
    qt = np.empty((N + 512, IW), np.int32)
    np.copyto(qt[:N], np.asarray(qweight).T)
    qt[N:] = 0
    lut64 = np.asarray(lut, dtype=np.float64)         # [N, 16]
    cf = np.einsum("my,ngy->ngm", _VINV, lut64.reshape(N, 4, 4))
    coef = np.zeros((N + 512, 16), np.float32)
    coef[:N] = cf.reshape(N, 16).astype(np.float32)
    return x2d, qt, coef


# ---------------- jit runner (built once per process) ---------------- #
_FN_CACHE = None


def _get_fn():
    global _FN_CACHE
    if _FN_CACHE is not None:
        return _FN_CACHE
    _install_neff_cache()
    nc = _get_nc()

    partition_name = nc.partition_id_tensor.name if nc.partition_id_tensor else None
    in_names, out_names, out_avals = [], [], []
    for alloc in nc.m.functions[0].allocations:
        if not isinstance(alloc, mybir.MemoryLocationSet):
            continue
        name = alloc.memorylocations[0].name
        if alloc.kind == "ExternalInput":
            if name != partition_name:
                in_names.append(name)
        elif alloc.kind == "ExternalOutput":
            out_names.append(name)
            out_avals.append(
                jax.core.ShapedArray(
                    tuple(alloc.tensor_shape), mybir.dt.np(alloc.dtype)
                )
            )
    assert in_names == ["qt", "coef"] and out_names == ["xy"], (in_names, out_names)
    all_in = in_names + out_names
    if partition_name is not None:
        all_in.append(partition_name)

    def _body(qt_a, coef_a, xy_a):
        operands = [qt_a, coef_a, xy_a]
        if partition_name is not None:
            operands.append(partition_id_tensor())
        outs = _bass_exec_p.bind(
            *operands,
            out_avals=tuple(out_avals),
            in_names=tuple(all_in),
            out_names=tuple(out_names),
            lowering_input_output_aliases=(),
            sim_require_finite=True,
            sim_require_nnan=True,
            nc=nc,
        )
        return outs[0]

    devices = jax.devices()[:NCORES]
    mesh = Mesh(np.asarray(devices), ("core",))
    _FN_CACHE = jax.jit(
        shard_map(
            _body,
            mesh=mesh,
            in_specs=(PartitionSpec(), PartitionSpec(), PartitionSpec("core")),
            out_specs=PartitionSpec("core"),
            check_rep=False,
        ),
        donate_argnums=(2,),
        keep_unused=True,
    )
    return _FN_CACHE


def kernel(input, qweight, lut):
    fn = _get_fn()
    x2d, qt, coef = _host_prep(input, qweight, lut)
    y = fn(qt, coef, x2d)
    _, to_f32 = _cpu_casts()
    with jax.default_device(_CPU):
        yf = np.asarray(to_f32(np.asarray(y)))
    return yf.reshape(B, S, N)

